# revision 12
# baseline (speedup 1.0000x reference)
"""Causal multi-head attention on 8 trn2 NeuronCores.

Problem: B=4, S=2048, D=2048, H=16 heads, head_dim=128, causal softmax,
torch-style Linear projections (W stored [in, out]).

Sharding: core c handles batch b = c//2 and head-group g = c%2
(8 heads = 1024 output columns of Wq/Wk/Wv, 1024 rows of Wo).
Each core produces a partial output [S, D]; host sums the two
head-group partials per batch and adds bo.

Per-core device pipeline (all matmuls fp32r, 1 cycle/row):
  Phase A: from xT (host-pretransposed [D, S]) compute
           Q^T, K^T [1024, S] and V [S, 1024]; spill to DRAM scratch.
  Phase B: per head h: scores^T tiles [128 k, 512 q] = K_h Q_h^T,
           causal mask (additive, precomputed), exp (no max-subtract:
           scores are O(5), fp32 exp is safe), ctx^T accumulation
           C^T = V_h^T-blocks @ P^T, denominators via ones-matmul,
           normalize with reciprocal broadcast (PE outer product).
  Phase C: out_partial = C @ Wo_slice via C^T blocks as lhsT.
"""

import numpy as np

import concourse.bass as bass
import concourse.mybir as mybir
import concourse.tile as tile
from concourse import bacc
from concourse.bass_utils import run_bass_kernel_spmd

B = 4
S = 2048
D = 2048
H = 16
DH = 128
HPC = 8          # heads per core
DHG = HPC * DH   # 1024: head-group width per core
KT = D // 128    # 16 k-tiles over the model dim
ST = S // 128    # 16 s-tiles
QC = S // 512    # 4 q-chunks
SCALE = 1.0 / np.sqrt(DH)
NEG = -1.0e30

F32 = mybir.dt.float32
F32R = mybir.dt.float32r


def _build_nc():
    nc = bacc.Bacc(None, target_bir_lowering=False)

    xT = nc.declare_dram_parameter("xT", [D, S], F32, isOutput=False)
    wq = nc.declare_dram_parameter("wq", [D, DHG], F32, isOutput=False)
    wk = nc.declare_dram_parameter("wk", [D, DHG], F32, isOutput=False)
    wv = nc.declare_dram_parameter("wv", [D, DHG], F32, isOutput=False)
    wo = nc.declare_dram_parameter("wo", [DHG, D], F32, isOutput=False)
    bqT = nc.declare_dram_parameter("bqT", [128, HPC], F32, isOutput=False)
    bkT = nc.declare_dram_parameter("bkT", [128, HPC], F32, isOutput=False)
    bvb = nc.declare_dram_parameter("bvb", [128, DHG], F32, isOutput=False)
    cmask = nc.declare_dram_parameter("cmask", [128, 896], F32, isOutput=False)
    out = nc.declare_dram_parameter("out", [S, D], F32, isOutput=True)

    with tile.TileContext(nc) as tc:
        _emit(nc, tc, xT, wq, wk, wv, wo, bqT, bkT, bvb, cmask, out)
    nc.compile()
    return nc


def _emit(nc, tc, xT, wq, wk, wv, wo, bqT, bkT, bvb, cmask, out):
    with (
        tc.tile_pool(name="const", bufs=1) as const,
        tc.tile_pool(name="dram", bufs=1, space="DRAM") as dram,
        tc.tile_pool(name="ct", bufs=1) as ctpool,
    ):
        qt_d = dram.tile([DHG, S], F32R)
        kt_d = dram.tile([DHG, S], F32R)
        v_d = dram.tile([S, DHG], F32R)

        cm_sb = const.tile([128, 896], F32)
        nc.sync.dma_start(out=cm_sb, in_=cmask[:, :])
        bq_sb = const.tile([128, HPC], F32)
        nc.sync.dma_start(out=bq_sb, in_=bqT[:, :])
        bk_sb = const.tile([128, HPC], F32)
        nc.sync.dma_start(out=bk_sb, in_=bkT[:, :])
        bv_sb = const.tile([128, DHG], F32)
        nc.sync.dma_start(out=bv_sb, in_=bvb[:, :])
        ones_f32 = const.tile([128, 128], F32)
        nc.vector.memset(ones_f32, 1.0)
        ones128 = const.tile([128, 128], F32R)
        nc.vector.tensor_copy(out=ones128, in_=ones_f32)

        # ---------------- Phase A: projections, spilled to DRAM -------------
        # Two S-halves so xT residency is 8MB instead of 16MB.
        wq_r = wq.bitcast(F32R).rearrange("(n p) m -> p n m", p=128)
        wk_r = wk.bitcast(F32R).rearrange("(n p) m -> p n m", p=128)
        wv_r = wv.bitcast(F32R).rearrange("(n p) m -> p n m", p=128)

        with (
            tc.tile_pool(name="xts", bufs=3) as xtp,
            tc.tile_pool(name="wqk", bufs=2) as wqk,
            tc.tile_pool(name="wvp", bufs=4) as wvp,
            tc.tile_pool(name="apsum", bufs=4, space="PSUM") as aps,
            tc.tile_pool(name="astage", bufs=4) as ast,
        ):
            for sh in range(2):
                s0 = sh * (S // 2)
                # xT half as two sub-tiles of 8 k-tiles each (bufs=3 lets the
                # next half's first sub-tile prefetch during this half).
                xt_lo = xtp.tile([128, 8, S // 2], F32R, tag="xts")
                xt_hi = xtp.tile([128, 8, S // 2], F32R, tag="xts")

                def xt_blk(kd):
                    t = xt_lo if kd < 8 else xt_hi
                    return t[:, kd % 8, :]

                for kd in range(KT):
                    nc.sync.dma_start(
                        out=xt_blk(kd),
                        in_=xT[kd * 128 : (kd + 1) * 128, s0 : s0 + S // 2].bitcast(F32R),
                    )

                # Q^T and K^T: psum[dh 128, s 512] = sum_kd Wblk^T @ xTblk
                for w_r, b_sb, dst in ((wq_r, bq_sb, qt_d), (wk_r, bk_sb, kt_d)):
                    for t in range(HPC):
                        w_sb = wqk.tile([128, KT, 128], F32R, tag="wqk")
                        nc.sync.dma_start(
                            out=w_sb, in_=w_r[:, :, t * 128 : (t + 1) * 128]
                        )
                        for sc in range(2):
                            psum = aps.tile([128, 512], F32)
                            for kd in range(KT):
                                nc.tensor.matmul(
                                    psum,
                                    w_sb[:, kd, :],
                                    xt_blk(kd)[:, sc * 512 : (sc + 1) * 512],
                                    start=(kd == 0),
                                    stop=(kd == KT - 1),
                                )
                            stg = ast.tile([128, 512], F32R, tag="astage")
                            nc.vector.tensor_scalar_add(
                                out=stg, in0=psum, scalar1=b_sb[:, t : t + 1]
                            )
                            nc.scalar.dma_start(
                                out=dst[
                                    t * 128 : (t + 1) * 128,
                                    s0 + sc * 512 : s0 + (sc + 1) * 512,
                                ],
                                in_=stg,
                            )

                # V: psum[s 128, dh 512] = sum_kd xTblk^T @ Wvblk.
                # wv streamed per k-tile; 4 s-tiles accumulate concurrently.
                for t2 in range(2):
                    for sb in range(2):  # blocks of 4 s-tiles
                        psums = [
                            aps.tile([128, 512], F32, tag="apsum", name=f"vps{si}")
                            for si in range(4)
                        ]
                        for kd in range(KT):
                            wv_sb = wvp.tile([128, 512], F32R, tag="wvp")
                            nc.sync.dma_start(
                                out=wv_sb,
                                in_=wv_r[:, kd, t2 * 512 : (t2 + 1) * 512],
                            )
                            for si in range(4):
                                st = sb * 4 + si
                                nc.tensor.matmul(
                                    psums[si],
                                    xt_blk(kd)[:, st * 128 : (st + 1) * 128],
                                    wv_sb,
                                    start=(kd == 0),
                                    stop=(kd == KT - 1),
                                )
                        for si in range(4):
                            st = sb * 4 + si
                            stg = ast.tile([128, 512], F32R, tag="astage")
                            nc.vector.tensor_tensor(
                                out=stg,
                                in0=psums[si],
                                in1=bv_sb[:, t2 * 512 : (t2 + 1) * 512],
                                op=mybir.AluOpType.add,
                            )
                            nc.scalar.dma_start(
                                out=v_d[
                                    s0 + st * 128 : s0 + (st + 1) * 128,
                                    t2 * 512 : (t2 + 1) * 512,
                                ],
                                in_=stg,
                            )

        # ---------------- Phase B: per-head attention ------------------------
        v_r = v_d[:, :].rearrange("(n p) d -> p n d", p=128)
        with (
            tc.tile_pool(name="qkv", bufs=2) as qkv,
            tc.tile_pool(name="ptile", bufs=6) as ppool,
            tc.tile_pool(name="msk", bufs=3) as mpool,
            tc.tile_pool(name="rcp", bufs=2) as rcpool,
            tc.tile_pool(name="pscore", bufs=3, space="PSUM") as pscore,
            tc.tile_pool(name="pctx", bufs=2, space="PSUM") as pctx,
            tc.tile_pool(name="psum2", bufs=2, space="PSUM") as psums,
        ):
            for h in range(HPC):
                qt_sb = qkv.tile([128, S], F32R, tag="qt")
                nc.sync.dma_start(out=qt_sb, in_=qt_d[h * 128 : (h + 1) * 128, :])
                kt_sb = qkv.tile([128, S], F32R, tag="kt")
                nc.sync.dma_start(out=kt_sb, in_=kt_d[h * 128 : (h + 1) * 128, :])
                v_sb = qkv.tile([128, ST, 128], F32R, tag="v")
                nc.sync.dma_start(out=v_sb, in_=v_r[:, :, h * 128 : (h + 1) * 128])

                for qc in range(QC):
                    nkt = 4 * qc + 4
                    # diagonal tiles first: their longer PE->DVE->ACT chains
                    # start early and overlap with the full tiles' stream
                    order = list(range(4 * qc, nkt)) + list(range(4 * qc))
                    psum_c = pctx.tile([128, 512], F32)
                    psum_s = psums.tile([128, 512], F32)

                    def scores(kt_i):
                        # diagonal tile j has valid columns only at qq >= 128j:
                        # compute just that [128, 512-128j] strip
                        j = kt_i - 4 * qc
                        off = 128 * j if j > 0 else 0
                        ps_t = pscore.tile([128, 512], F32, tag="ps_t")
                        nc.tensor.matmul(
                            ps_t[:, off:],
                            kt_sb[:, kt_i * 128 : (kt_i + 1) * 128],
                            qt_sb[:, qc * 512 + off : (qc + 1) * 512],
                            start=True,
                            stop=True,
                        )
                        p_t = ppool.tile([128, 512], F32R, tag="p_t")
                        if j >= 0:
                            msk = mpool.tile([128, 512], F32, tag="msk")
                            nc.vector.tensor_tensor(
                                out=msk[:, off:],
                                in0=ps_t[:, off:],
                                in1=cm_sb[:, 384 : 896 - off],
                                op=mybir.AluOpType.add,
                            )
                            src = msk
                        else:
                            src = ps_t
                        nc.scalar.activation(
                            out=p_t[:, off:],
                            in_=src[:, off:],
                            func=mybir.ActivationFunctionType.Exp,
                            scale=float(SCALE),
                        )
                        return p_t, off

                    def ctx(idx, kt_i, p_t, off):
                        nc.tensor.matmul(
                            psum_c[:, off:],
                            v_sb[:, kt_i, :],
                            p_t[:, off:],
                            start=(idx == 0),
                            stop=(idx == nkt - 1),
                        )
                        # every psum_s row accumulates the per-q denominator
                        nc.tensor.matmul(
                            psum_s[:, off:],
                            ones128,
                            p_t[:, off:],
                            start=(idx == 0),
                            stop=(idx == nkt - 1),
                        )

                    # software-pipeline scores/exp ahead of ctx by one tile
                    prev = None
                    for idx, kt_i in enumerate(order):
                        p_t, off = scores(kt_i)
                        if prev is not None:
                            ctx(idx - 1, prev[0], prev[1], prev[2])
                        prev = (kt_i, p_t, off)
                    ctx(nkt - 1, prev[0], prev[1], prev[2])

                    recip = rcpool.tile([128, 512], F32, tag="rcp")
                    nc.vector.reciprocal_approx_fast(out=recip, in_=psum_s)
                    ct = ctpool.tile([128, 512], F32R, tag=f"ct_{h}_{qc}")
                    nc.vector.tensor_tensor(
                        out=ct,
                        in0=psum_c,
                        in1=recip,
                        op=mybir.AluOpType.mult,
                    )
                    ctpool_tiles[(h, qc)] = ct

        # ---------------- Phase C: output projection -------------------------
        wo_r = wo.bitcast(F32R).rearrange("(n p) m -> p n m", p=128)
        with (
            tc.tile_pool(name="wop", bufs=1) as wop,
            tc.tile_pool(name="opsum", bufs=4, space="PSUM") as ops,
            tc.tile_pool(name="ostage", bufs=4) as ost,
        ):
            wo_sb = wop.tile([128, HPC, D], F32R)
            for hh in range(HPC):
                nc.sync.dma_start(out=wo_sb[:, hh, :], in_=wo_r[:, hh, :])

            for st in range(ST):
                qc = st // 4
                off = (st % 4) * 128
                for ncol in range(4):
                    psum = ops.tile([128, 512], F32)
                    for hh in range(HPC):
                        ct = ctpool_tiles[(hh, qc)]
                        nc.tensor.matmul(
                            psum,
                            ct[:, off : off + 128],
                            wo_sb[:, hh, ncol * 512 : (ncol + 1) * 512],
                            start=(hh == 0),
                            stop=(hh == HPC - 1),
                        )
                    o_sb = ost.tile([128, 512], F32, tag="ostage")
                    nc.scalar.activation(
                        out=o_sb, in_=psum, func=mybir.ActivationFunctionType.Copy
                    )
                    nc.scalar.dma_start(
                        out=out[
                            st * 128 : (st + 1) * 128,
                            ncol * 512 : (ncol + 1) * 512,
                        ],
                        in_=o_sb,
                    )


ctpool_tiles = {}

_NC = None


def _get_nc():
    global _NC
    if _NC is None:
        ctpool_tiles.clear()
        _NC = _build_nc()
    return _NC


def _host_prep(input_sequences, Wq, bq, Wk, bk, Wv, bv, Wo, bo):
    """Build per-core input maps."""
    x = np.asarray(input_sequences, dtype=np.float32)
    cm = np.full((128, 896), NEG, dtype=np.float32)
    kk = np.arange(128)[:, None]
    uu = np.arange(896)[None, :]
    cm[kk <= uu - 384] = 0.0

    in_maps = []
    for c in range(8):
        b, g = divmod(c, 2)
        sl = slice(g * DHG, (g + 1) * DHG)
        wq_c = np.ascontiguousarray(Wq[:, sl], dtype=np.float32)
        wk_c = np.ascontiguousarray(Wk[:, sl], dtype=np.float32)
        wv_c = np.ascontiguousarray(Wv[:, sl], dtype=np.float32)
        wo_c = np.ascontiguousarray(Wo[sl, :], dtype=np.float32)
        in_maps.append({
            "xT": np.ascontiguousarray(x[b].T),
            "wq": wq_c,
            "wk": wk_c,
            "wv": wv_c,
            "wo": wo_c,
            "bqT": np.ascontiguousarray(
                np.asarray(bq[sl], dtype=np.float32).reshape(HPC, 128).T
            ),
            "bkT": np.ascontiguousarray(
                np.asarray(bk[sl], dtype=np.float32).reshape(HPC, 128).T
            ),
            "bvb": np.ascontiguousarray(
                np.broadcast_to(np.asarray(bv[sl], dtype=np.float32), (128, DHG))
            ),
            "cmask": cm,
        })
    return in_maps


def kernel(input_sequences, Wq, bq, Wk, bk, Wv, bv, Wo, bo, _trace=False):
    nc = _get_nc()
    in_maps = _host_prep(input_sequences, Wq, bq, Wk, bk, Wv, bv, Wo, bo)
    res = run_bass_kernel_spmd(nc, in_maps, list(range(8)), trace=_trace)
    bo32 = np.asarray(bo, dtype=np.float32)
    out = np.empty((B, S, D), dtype=np.float32)
    for b in range(B):
        out[b] = res.results[2 * b]["out"] + res.results[2 * b + 1]["out"] + bo32
    if _trace:
        kernel.last_exec_time_ns = res.exec_time_ns
    return out


# revision 13
# speedup vs baseline: 1.1319x; 1.1319x over previous
"""Causal multi-head attention on 8 trn2 NeuronCores.

Problem: B=4, S=2048, D=2048, H=16 heads, head_dim=128, causal softmax,
torch-style Linear projections (W stored [in, out]).

Sharding: core c handles batch b = c//2 and head-group g = c%2
(8 heads = 1024 output columns of Wq/Wk/Wv, 1024 rows of Wo).
Each core produces a partial output [S, D]; host sums the two
head-group partials per batch and adds bo.

Per-core device pipeline (all matmuls fp32r, 1 cycle/row):
  Phase A: from xT (host-pretransposed [D, S]) compute
           Q^T, K^T [1024, S] and V [S, 1024]; spill to DRAM scratch.
  Phase B: per head h: scores^T tiles [128 k, 512 q] = K_h Q_h^T,
           causal mask (additive, precomputed), exp (no max-subtract:
           scores are O(5), fp32 exp is safe), ctx^T accumulation
           C^T = V_h^T-blocks @ P^T, denominators via ones-matmul,
           normalize with reciprocal broadcast (PE outer product).
  Phase C: out_partial = C @ Wo_slice via C^T blocks as lhsT.
"""

import numpy as np

import concourse.bass as bass
import concourse.mybir as mybir
import concourse.tile as tile
from concourse import bacc
from concourse.bass_utils import run_bass_kernel_spmd

B = 4
S = 2048
D = 2048
H = 16
DH = 128
HPC = 8          # heads per core
DHG = HPC * DH   # 1024: head-group width per core
KT = D // 128    # 16 k-tiles over the model dim
ST = S // 128    # 16 s-tiles
QC = S // 512    # 4 q-chunks
SCALE = 1.0 / np.sqrt(DH)
NEG = -1.0e30

F32 = mybir.dt.float32
F32R = mybir.dt.float32r


def _build_nc():
    nc = bacc.Bacc(None, target_bir_lowering=False)

    xT = nc.declare_dram_parameter("xT", [D, S], F32, isOutput=False)
    wq = nc.declare_dram_parameter("wq", [D, DHG], F32, isOutput=False)
    wk = nc.declare_dram_parameter("wk", [D, DHG], F32, isOutput=False)
    wv = nc.declare_dram_parameter("wv", [D, DHG], F32, isOutput=False)
    wo = nc.declare_dram_parameter("wo", [DHG, D], F32, isOutput=False)
    bqT = nc.declare_dram_parameter("bqT", [128, HPC], F32, isOutput=False)
    bkT = nc.declare_dram_parameter("bkT", [128, HPC], F32, isOutput=False)
    bvb = nc.declare_dram_parameter("bvb", [128, DHG], F32, isOutput=False)
    cmask = nc.declare_dram_parameter("cmask", [128, 896], F32, isOutput=False)
    out = nc.declare_dram_parameter("out", [S, D], F32, isOutput=True)

    with tile.TileContext(nc) as tc:
        _emit(nc, tc, xT, wq, wk, wv, wo, bqT, bkT, bvb, cmask, out)
    nc.compile()
    return nc


def _emit(nc, tc, xT, wq, wk, wv, wo, bqT, bkT, bvb, cmask, out):
    with (
        tc.tile_pool(name="const", bufs=1) as const,
        tc.tile_pool(name="dram", bufs=1, space="DRAM") as dram,
        tc.tile_pool(name="ct", bufs=1) as ctpool,
    ):
        qt_d = dram.tile([DHG, S], F32R)
        kt_d = dram.tile([DHG, S], F32R)
        v_d = dram.tile([S, DHG], F32R)

        cm_sb = const.tile([128, 896], F32)
        nc.sync.dma_start(out=cm_sb, in_=cmask[:, :])
        bq_sb = const.tile([128, HPC], F32)
        nc.sync.dma_start(out=bq_sb, in_=bqT[:, :])
        bk_sb = const.tile([128, HPC], F32)
        nc.sync.dma_start(out=bk_sb, in_=bkT[:, :])
        bv_sb = const.tile([128, DHG], F32)
        nc.sync.dma_start(out=bv_sb, in_=bvb[:, :])
        ones_f32 = const.tile([128, 128], F32)
        nc.vector.memset(ones_f32, 1.0)
        ones128 = const.tile([128, 128], F32R)
        nc.vector.tensor_copy(out=ones128, in_=ones_f32)

        # ---------------- Phase A: projections, spilled to DRAM -------------
        # Two S-halves so xT residency is 8MB instead of 16MB.
        wq_r = wq.bitcast(F32R).rearrange("(n p) m -> p n m", p=128)
        wk_r = wk.bitcast(F32R).rearrange("(n p) m -> p n m", p=128)
        wv_r = wv.bitcast(F32R).rearrange("(n p) m -> p n m", p=128)

        with (
            tc.tile_pool(name="xts", bufs=3) as xtp,
            tc.tile_pool(name="wqk", bufs=2) as wqk,
            tc.tile_pool(name="wvp", bufs=4) as wvp,
            tc.tile_pool(name="apsum", bufs=4, space="PSUM") as aps,
            tc.tile_pool(name="astage", bufs=4) as ast,
        ):
            for sh in range(2):
                s0 = sh * (S // 2)
                # xT half as two sub-tiles of 8 k-tiles each (bufs=3 lets the
                # next half's first sub-tile prefetch during this half).
                xt_lo = xtp.tile([128, 8, S // 2], F32R, tag="xts")
                xt_hi = xtp.tile([128, 8, S // 2], F32R, tag="xts")

                def xt_blk(kd):
                    t = xt_lo if kd < 8 else xt_hi
                    return t[:, kd % 8, :]

                for kd in range(KT):
                    nc.sync.dma_start(
                        out=xt_blk(kd),
                        in_=xT[kd * 128 : (kd + 1) * 128, s0 : s0 + S // 2].bitcast(F32R),
                    )

                # Q^T and K^T: psum[dh 128, s 512] = sum_kd Wblk^T @ xTblk
                for w_r, b_sb, dst in ((wq_r, bq_sb, qt_d), (wk_r, bk_sb, kt_d)):
                    for t in range(HPC):
                        w_sb = wqk.tile([128, KT, 128], F32R, tag="wqk")
                        nc.sync.dma_start(
                            out=w_sb, in_=w_r[:, :, t * 128 : (t + 1) * 128]
                        )
                        for sc in range(2):
                            psum = aps.tile([128, 512], F32)
                            for kd in range(KT):
                                nc.tensor.matmul(
                                    psum,
                                    w_sb[:, kd, :],
                                    xt_blk(kd)[:, sc * 512 : (sc + 1) * 512],
                                    start=(kd == 0),
                                    stop=(kd == KT - 1),
                                )
                            stg = ast.tile([128, 512], F32R, tag="astage")
                            nc.vector.tensor_scalar_add(
                                out=stg, in0=psum, scalar1=b_sb[:, t : t + 1]
                            )
                            nc.sync.dma_start(
                                out=dst[
                                    t * 128 : (t + 1) * 128,
                                    s0 + sc * 512 : s0 + (sc + 1) * 512,
                                ],
                                in_=stg,
                            )

                # V: psum[s 128, dh 512] = sum_kd xTblk^T @ Wvblk.
                # wv streamed per k-tile; 4 s-tiles accumulate concurrently.
                for t2 in range(2):
                    for sb in range(2):  # blocks of 4 s-tiles
                        psums = [
                            aps.tile([128, 512], F32, tag="apsum", name=f"vps{si}")
                            for si in range(4)
                        ]
                        for kd in range(KT):
                            wv_sb = wvp.tile([128, 512], F32R, tag="wvp")
                            nc.sync.dma_start(
                                out=wv_sb,
                                in_=wv_r[:, kd, t2 * 512 : (t2 + 1) * 512],
                            )
                            for si in range(4):
                                st = sb * 4 + si
                                nc.tensor.matmul(
                                    psums[si],
                                    xt_blk(kd)[:, st * 128 : (st + 1) * 128],
                                    wv_sb,
                                    start=(kd == 0),
                                    stop=(kd == KT - 1),
                                )
                        for si in range(4):
                            st = sb * 4 + si
                            stg = ast.tile([128, 512], F32R, tag="astage")
                            nc.vector.tensor_tensor(
                                out=stg,
                                in0=psums[si],
                                in1=bv_sb[:, t2 * 512 : (t2 + 1) * 512],
                                op=mybir.AluOpType.add,
                            )
                            nc.sync.dma_start(
                                out=v_d[
                                    s0 + st * 128 : s0 + (st + 1) * 128,
                                    t2 * 512 : (t2 + 1) * 512,
                                ],
                                in_=stg,
                            )

        # ---------------- Phase B: per-head attention ------------------------
        v_r = v_d[:, :].rearrange("(n p) d -> p n d", p=128)
        with (
            tc.tile_pool(name="qkv", bufs=2) as qkv,
            tc.tile_pool(name="ptile", bufs=6) as ppool,
            tc.tile_pool(name="msk", bufs=3) as mpool,
            tc.tile_pool(name="rcp", bufs=2) as rcpool,
            tc.tile_pool(name="pscore", bufs=3, space="PSUM") as pscore,
            tc.tile_pool(name="pctx", bufs=2, space="PSUM") as pctx,
            tc.tile_pool(name="psum2", bufs=2, space="PSUM") as psums,
        ):
            for h in range(HPC):
                qt_sb = qkv.tile([128, S], F32R, tag="qt")
                nc.sync.dma_start(out=qt_sb, in_=qt_d[h * 128 : (h + 1) * 128, :])
                kt_sb = qkv.tile([128, S], F32R, tag="kt")
                nc.sync.dma_start(out=kt_sb, in_=kt_d[h * 128 : (h + 1) * 128, :])
                v_sb = qkv.tile([128, ST, 128], F32R, tag="v")
                nc.sync.dma_start(out=v_sb, in_=v_r[:, :, h * 128 : (h + 1) * 128])

                for qc in range(QC):
                    nkt = 4 * qc + 4
                    # diagonal tiles first: their longer PE->DVE->ACT chains
                    # start early and overlap with the full tiles' stream
                    order = list(range(4 * qc, nkt)) + list(range(4 * qc))
                    psum_c = pctx.tile([128, 512], F32)
                    psum_s = psums.tile([128, 512], F32)

                    def scores(kt_i):
                        # diagonal tile j has valid columns only at qq >= 128j:
                        # compute just that [128, 512-128j] strip
                        j = kt_i - 4 * qc
                        off = 128 * j if j > 0 else 0
                        ps_t = pscore.tile([128, 512], F32, tag="ps_t")
                        nc.tensor.matmul(
                            ps_t[:, off:],
                            kt_sb[:, kt_i * 128 : (kt_i + 1) * 128],
                            qt_sb[:, qc * 512 + off : (qc + 1) * 512],
                            start=True,
                            stop=True,
                        )
                        p_t = ppool.tile([128, 512], F32R, tag="p_t")
                        if j >= 0:
                            msk = mpool.tile([128, 512], F32, tag="msk")
                            nc.vector.tensor_tensor(
                                out=msk[:, off:],
                                in0=ps_t[:, off:],
                                in1=cm_sb[:, 384 : 896 - off],
                                op=mybir.AluOpType.add,
                            )
                            src = msk
                        else:
                            src = ps_t
                        nc.scalar.activation(
                            out=p_t[:, off:],
                            in_=src[:, off:],
                            func=mybir.ActivationFunctionType.Exp,
                            scale=float(SCALE),
                        )
                        return p_t, off

                    def ctx(idx, kt_i, p_t, off):
                        nc.tensor.matmul(
                            psum_c[:, off:],
                            v_sb[:, kt_i, :],
                            p_t[:, off:],
                            start=(idx == 0),
                            stop=(idx == nkt - 1),
                        )
                        # every psum_s row accumulates the per-q denominator
                        nc.tensor.matmul(
                            psum_s[:, off:],
                            ones128,
                            p_t[:, off:],
                            start=(idx == 0),
                            stop=(idx == nkt - 1),
                        )

                    # software-pipeline scores/exp ahead of ctx by one tile
                    prev = None
                    for idx, kt_i in enumerate(order):
                        p_t, off = scores(kt_i)
                        if prev is not None:
                            ctx(idx - 1, prev[0], prev[1], prev[2])
                        prev = (kt_i, p_t, off)
                    ctx(nkt - 1, prev[0], prev[1], prev[2])

                    recip = rcpool.tile([128, 512], F32, tag="rcp")
                    nc.vector.reciprocal_approx_fast(out=recip, in_=psum_s)
                    ct = ctpool.tile([128, 512], F32R, tag=f"ct_{h}_{qc}")
                    nc.vector.tensor_tensor(
                        out=ct,
                        in0=psum_c,
                        in1=recip,
                        op=mybir.AluOpType.mult,
                    )
                    ctpool_tiles[(h, qc)] = ct

        # ---------------- Phase C: output projection -------------------------
        wo_r = wo.bitcast(F32R).rearrange("(n p) m -> p n m", p=128)
        with (
            tc.tile_pool(name="wop", bufs=1) as wop,
            tc.tile_pool(name="opsum", bufs=4, space="PSUM") as ops,
            tc.tile_pool(name="ostage", bufs=4) as ost,
        ):
            wo_sb = wop.tile([128, HPC, D], F32R)
            for hh in range(HPC):
                nc.sync.dma_start(out=wo_sb[:, hh, :], in_=wo_r[:, hh, :])

            for st in range(ST):
                qc = st // 4
                off = (st % 4) * 128
                for ncol in range(4):
                    psum = ops.tile([128, 512], F32)
                    for hh in range(HPC):
                        ct = ctpool_tiles[(hh, qc)]
                        nc.tensor.matmul(
                            psum,
                            ct[:, off : off + 128],
                            wo_sb[:, hh, ncol * 512 : (ncol + 1) * 512],
                            start=(hh == 0),
                            stop=(hh == HPC - 1),
                        )
                    o_sb = ost.tile([128, 512], F32, tag="ostage")
                    nc.scalar.activation(
                        out=o_sb, in_=psum, func=mybir.ActivationFunctionType.Copy
                    )
                    nc.sync.dma_start(
                        out=out[
                            st * 128 : (st + 1) * 128,
                            ncol * 512 : (ncol + 1) * 512,
                        ],
                        in_=o_sb,
                    )


ctpool_tiles = {}

_NC = None


def _get_nc():
    global _NC
    if _NC is None:
        ctpool_tiles.clear()
        _NC = _build_nc()
    return _NC


def _host_prep(input_sequences, Wq, bq, Wk, bk, Wv, bv, Wo, bo):
    """Build per-core input maps."""
    x = np.asarray(input_sequences, dtype=np.float32)
    cm = np.full((128, 896), NEG, dtype=np.float32)
    kk = np.arange(128)[:, None]
    uu = np.arange(896)[None, :]
    cm[kk <= uu - 384] = 0.0

    in_maps = []
    for c in range(8):
        b, g = divmod(c, 2)
        sl = slice(g * DHG, (g + 1) * DHG)
        wq_c = np.ascontiguousarray(Wq[:, sl], dtype=np.float32)
        wk_c = np.ascontiguousarray(Wk[:, sl], dtype=np.float32)
        wv_c = np.ascontiguousarray(Wv[:, sl], dtype=np.float32)
        wo_c = np.ascontiguousarray(Wo[sl, :], dtype=np.float32)
        in_maps.append({
            "xT": np.ascontiguousarray(x[b].T),
            "wq": wq_c,
            "wk": wk_c,
            "wv": wv_c,
            "wo": wo_c,
            "bqT": np.ascontiguousarray(
                np.asarray(bq[sl], dtype=np.float32).reshape(HPC, 128).T
            ),
            "bkT": np.ascontiguousarray(
                np.asarray(bk[sl], dtype=np.float32).reshape(HPC, 128).T
            ),
            "bvb": np.ascontiguousarray(
                np.broadcast_to(np.asarray(bv[sl], dtype=np.float32), (128, DHG))
            ),
            "cmask": cm,
        })
    return in_maps


def kernel(input_sequences, Wq, bq, Wk, bk, Wv, bv, Wo, bo, _trace=False):
    nc = _get_nc()
    in_maps = _host_prep(input_sequences, Wq, bq, Wk, bk, Wv, bv, Wo, bo)
    res = run_bass_kernel_spmd(nc, in_maps, list(range(8)), trace=_trace)
    bo32 = np.asarray(bo, dtype=np.float32)
    out = np.empty((B, S, D), dtype=np.float32)
    for b in range(B):
        out[b] = res.results[2 * b]["out"] + res.results[2 * b + 1]["out"] + bo32
    if _trace:
        kernel.last_exec_time_ns = res.exec_time_ns
    return out


# revision 17
# speedup vs baseline: 1.1510x; 1.0169x over previous
"""Causal multi-head attention on 8 trn2 NeuronCores.

Problem: B=4, S=2048, D=2048, H=16 heads, head_dim=128, causal softmax,
torch-style Linear projections (W stored [in, out]).

Sharding: core c handles batch b = c//2 and head-group g = c%2
(8 heads = 1024 output columns of Wq/Wk/Wv, 1024 rows of Wo).
Each core produces a partial output [S, D]; host sums the two
head-group partials per batch and adds bo.

Per-core device pipeline (all matmuls fp32r, 1 cycle/row):
  Phase A: from xT (host-pretransposed [D, S]) compute
           Q^T, K^T [1024, S] and V [S, 1024]; spill to DRAM scratch.
  Phase B: per head h: scores^T tiles [128 k, 512 q] = K_h Q_h^T,
           causal mask (additive, precomputed), exp (no max-subtract:
           scores are O(5), fp32 exp is safe), ctx^T accumulation
           C^T = V_h^T-blocks @ P^T, denominators via ones-matmul,
           normalize with reciprocal broadcast (PE outer product).
  Phase C: out_partial = C @ Wo_slice via C^T blocks as lhsT.
"""

import numpy as np

import concourse.bass as bass
import concourse.mybir as mybir
import concourse.tile as tile
from concourse import bacc
from concourse.bass_utils import run_bass_kernel_spmd

B = 4
S = 2048
D = 2048
H = 16
DH = 128
HPC = 8          # heads per core
DHG = HPC * DH   # 1024: head-group width per core
KT = D // 128    # 16 k-tiles over the model dim
ST = S // 128    # 16 s-tiles
QC = S // 512    # 4 q-chunks
SCALE = 1.0 / np.sqrt(DH)
NEG = -1.0e30

F32 = mybir.dt.float32
F32R = mybir.dt.float32r


def _build_nc():
    nc = bacc.Bacc(None, target_bir_lowering=False)

    xT = nc.declare_dram_parameter("xT", [D, S], F32, isOutput=False)
    # wq/wk host-pregathered to [HPC*128, KT*128]: row t*128+p, col n*128+m
    # = Wq[n*128+p, t*128+m] so each head-tile's weights DMA contiguously
    wq = nc.declare_dram_parameter("wq", [DHG, D], F32, isOutput=False)
    wk = nc.declare_dram_parameter("wk", [DHG, D], F32, isOutput=False)
    wv = nc.declare_dram_parameter("wv", [D, DHG], F32, isOutput=False)
    wo = nc.declare_dram_parameter("wo", [DHG, D], F32, isOutput=False)
    bqT = nc.declare_dram_parameter("bqT", [128, HPC], F32, isOutput=False)
    bkT = nc.declare_dram_parameter("bkT", [128, HPC], F32, isOutput=False)
    bvb = nc.declare_dram_parameter("bvb", [128, DHG], F32, isOutput=False)
    cmask = nc.declare_dram_parameter("cmask", [128, 896], F32, isOutput=False)
    out = nc.declare_dram_parameter("out", [S, D], F32, isOutput=True)

    with tile.TileContext(nc) as tc:
        _emit(nc, tc, xT, wq, wk, wv, wo, bqT, bkT, bvb, cmask, out)
    nc.compile()
    return nc


def _emit(nc, tc, xT, wq, wk, wv, wo, bqT, bkT, bvb, cmask, out):
    with (
        tc.tile_pool(name="const", bufs=1) as const,
        tc.tile_pool(name="dram", bufs=1, space="DRAM") as dram,
        tc.tile_pool(name="ct", bufs=4) as ctpool,
    ):
        qt_d = dram.tile([DHG, S], F32R)
        kt_d = dram.tile([DHG, S], F32R)
        v_d = dram.tile([S, DHG], F32R)
        ct_d = dram.tile([DHG, S], F32R)

        cm_sb = const.tile([128, 896], F32)
        nc.sync.dma_start(out=cm_sb, in_=cmask[:, :])
        bq_sb = const.tile([128, HPC], F32)
        nc.sync.dma_start(out=bq_sb, in_=bqT[:, :])
        bk_sb = const.tile([128, HPC], F32)
        nc.sync.dma_start(out=bk_sb, in_=bkT[:, :])
        bv_sb = const.tile([128, DHG], F32)
        nc.sync.dma_start(out=bv_sb, in_=bvb[:, :])
        ones_f32 = const.tile([128, 128], F32)
        nc.vector.memset(ones_f32, 1.0)
        ones128 = const.tile([128, 128], F32R)
        nc.vector.tensor_copy(out=ones128, in_=ones_f32)

        # ---------------- Phase A: projections, spilled to DRAM -------------
        # Two S-halves so xT residency is 8MB instead of 16MB.
        wv_r = wv.bitcast(F32R).rearrange("(n p) m -> p n m", p=128)

        with (
            tc.tile_pool(name="xts", bufs=3) as xtp,
            tc.tile_pool(name="wqk", bufs=3) as wqk,
            tc.tile_pool(name="wvp", bufs=4) as wvp,
            tc.tile_pool(name="apsum", bufs=4, space="PSUM") as aps,
            tc.tile_pool(name="astage", bufs=4) as ast,
        ):
            for sh in range(2):
                s0 = sh * (S // 2)
                # weight tiles prefetched (depth 2) ahead of the bulk xT DMAs
                seq = [(w, b, dst, t)
                       for w, b, dst in ((wq, bq_sb, qt_d), (wk, bk_sb, kt_d))
                       for t in range(HPC)]
                w_tiles = {}

                def w_prefetch(i):
                    if i < len(seq):
                        w, _, _, t = seq[i]
                        w_sb = wqk.tile([128, KT, 128], F32R, tag="wqk",
                                        name=f"w_sb{i % 3}")
                        nc.sync.dma_start(
                            out=w_sb,
                            in_=w[t * 128 : (t + 1) * 128, :]
                            .rearrange("p (n m) -> p n m", m=128)
                            .bitcast(F32R),
                        )
                        w_tiles[i] = w_sb

                w_prefetch(0)
                w_prefetch(1)

                # xT half as two sub-tiles of 8 k-tiles each (bufs=3 lets the
                # next half's first sub-tile prefetch during this half).
                xt_lo = xtp.tile([128, 8, S // 2], F32R, tag="xts")
                xt_hi = xtp.tile([128, 8, S // 2], F32R, tag="xts")

                def xt_blk(kd):
                    t = xt_lo if kd < 8 else xt_hi
                    return t[:, kd % 8, :]

                for kd in range(KT):
                    nc.sync.dma_start(
                        out=xt_blk(kd),
                        in_=xT[kd * 128 : (kd + 1) * 128, s0 : s0 + S // 2].bitcast(F32R),
                    )

                # Q^T and K^T: psum[dh 128, s 512] = sum_kd Wblk^T @ xTblk
                for i, (w, b_sb, dst, t) in enumerate(seq):
                    w_sb = w_tiles.pop(i)
                    w_prefetch(i + 2)
                    for sc in range(2):
                        psum = aps.tile([128, 512], F32)
                        for kd in range(KT):
                            nc.tensor.matmul(
                                psum,
                                w_sb[:, kd, :],
                                xt_blk(kd)[:, sc * 512 : (sc + 1) * 512],
                                start=(kd == 0),
                                stop=(kd == KT - 1),
                            )
                        stg = ast.tile([128, 512], F32R, tag="astage")
                        nc.vector.tensor_scalar_add(
                            out=stg, in0=psum, scalar1=b_sb[:, t : t + 1]
                        )
                        nc.gpsimd.dma_start(
                            out=dst[
                                t * 128 : (t + 1) * 128,
                                s0 + sc * 512 : s0 + (sc + 1) * 512,
                            ],
                            in_=stg,
                        )

                # V: psum[s 128, dh 512] = sum_kd xTblk^T @ Wvblk.
                # wv streamed per k-tile; 4 s-tiles accumulate concurrently.
                for t2 in range(2):
                    for sb in range(2):  # blocks of 4 s-tiles
                        psums = [
                            aps.tile([128, 512], F32, tag="apsum", name=f"vps{si}")
                            for si in range(4)
                        ]
                        for kd in range(KT):
                            wv_sb = wvp.tile([128, 512], F32R, tag="wvp")
                            nc.sync.dma_start(
                                out=wv_sb,
                                in_=wv_r[:, kd, t2 * 512 : (t2 + 1) * 512],
                            )
                            for si in range(4):
                                st = sb * 4 + si
                                nc.tensor.matmul(
                                    psums[si],
                                    xt_blk(kd)[:, st * 128 : (st + 1) * 128],
                                    wv_sb,
                                    start=(kd == 0),
                                    stop=(kd == KT - 1),
                                )
                        for si in range(4):
                            st = sb * 4 + si
                            stg = ast.tile([128, 512], F32R, tag="astage")
                            nc.vector.tensor_tensor(
                                out=stg,
                                in0=psums[si],
                                in1=bv_sb[:, t2 * 512 : (t2 + 1) * 512],
                                op=mybir.AluOpType.add,
                            )
                            nc.gpsimd.dma_start(
                                out=v_d[
                                    s0 + st * 128 : s0 + (st + 1) * 128,
                                    t2 * 512 : (t2 + 1) * 512,
                                ],
                                in_=stg,
                            )

        # ---------------- Phase B: per-head attention ------------------------
        v_r = v_d[:, :].rearrange("(n p) d -> p n d", p=128)
        with (
            tc.tile_pool(name="qkv", bufs=2) as qkv,
            tc.tile_pool(name="ptile", bufs=6) as ppool,
            tc.tile_pool(name="msk", bufs=3) as mpool,
            tc.tile_pool(name="rcp", bufs=2) as rcpool,
            tc.tile_pool(name="pscore", bufs=3, space="PSUM") as pscore,
            tc.tile_pool(name="pctx", bufs=2, space="PSUM") as pctx,
            tc.tile_pool(name="psum2", bufs=2, space="PSUM") as psums,
        ):
            for h in range(HPC):
                qt_sb = qkv.tile([128, S], F32R, tag="qt")
                nc.sync.dma_start(out=qt_sb, in_=qt_d[h * 128 : (h + 1) * 128, :])
                kt_sb = qkv.tile([128, S], F32R, tag="kt")
                nc.sync.dma_start(out=kt_sb, in_=kt_d[h * 128 : (h + 1) * 128, :])
                v_sb = qkv.tile([128, ST, 128], F32R, tag="v")
                nc.sync.dma_start(out=v_sb, in_=v_r[:, :, h * 128 : (h + 1) * 128])

                for qc in range(QC):
                    nkt = 4 * qc + 4
                    # diagonal tiles first: their longer PE->DVE->ACT chains
                    # start early and overlap with the full tiles' stream
                    order = list(range(4 * qc, nkt)) + list(range(4 * qc))
                    psum_c = pctx.tile([128, 512], F32)
                    psum_s = psums.tile([128, 512], F32)

                    def scores(kt_i):
                        # diagonal tile j has valid columns only at qq >= 128j:
                        # compute just that [128, 512-128j] strip
                        j = kt_i - 4 * qc
                        off = 128 * j if j > 0 else 0
                        ps_t = pscore.tile([128, 512], F32, tag="ps_t")
                        nc.tensor.matmul(
                            ps_t[:, off:],
                            kt_sb[:, kt_i * 128 : (kt_i + 1) * 128],
                            qt_sb[:, qc * 512 + off : (qc + 1) * 512],
                            start=True,
                            stop=True,
                        )
                        p_t = ppool.tile([128, 512], F32R, tag="p_t")
                        if j >= 0:
                            msk = mpool.tile([128, 512], F32, tag="msk")
                            nc.vector.tensor_tensor(
                                out=msk[:, off:],
                                in0=ps_t[:, off:],
                                in1=cm_sb[:, 384 : 896 - off],
                                op=mybir.AluOpType.add,
                            )
                            src = msk
                        else:
                            src = ps_t
                        nc.scalar.activation(
                            out=p_t[:, off:],
                            in_=src[:, off:],
                            func=mybir.ActivationFunctionType.Exp,
                            scale=float(SCALE),
                        )
                        return p_t, off

                    def ctx(idx, kt_i, p_t, off):
                        nc.tensor.matmul(
                            psum_c[:, off:],
                            v_sb[:, kt_i, :],
                            p_t[:, off:],
                            start=(idx == 0),
                            stop=(idx == nkt - 1),
                        )
                        # every psum_s row accumulates the per-q denominator
                        nc.tensor.matmul(
                            psum_s[:, off:],
                            ones128,
                            p_t[:, off:],
                            start=(idx == 0),
                            stop=(idx == nkt - 1),
                        )

                    # software-pipeline scores/exp ahead of ctx by one tile
                    prev = None
                    for idx, kt_i in enumerate(order):
                        p_t, off = scores(kt_i)
                        if prev is not None:
                            ctx(idx - 1, prev[0], prev[1], prev[2])
                        prev = (kt_i, p_t, off)
                    ctx(nkt - 1, prev[0], prev[1], prev[2])

                    recip = rcpool.tile([128, 512], F32, tag="rcp")
                    nc.vector.reciprocal_approx_fast(out=recip, in_=psum_s)
                    ct = ctpool.tile([128, 512], F32R, tag="ct")
                    nc.vector.tensor_tensor(
                        out=ct,
                        in0=psum_c,
                        in1=recip,
                        op=mybir.AluOpType.mult,
                    )
                    nc.gpsimd.dma_start(
                        out=ct_d[h * 128 : (h + 1) * 128, qc * 512 : (qc + 1) * 512],
                        in_=ct,
                    )

        # ---------------- Phase C: output projection -------------------------
        wo_r = wo.bitcast(F32R).rearrange("(n p) m -> p n m", p=128)
        ct_r = ct_d[:, :].rearrange("(n p) m -> p n m", p=128)
        with (
            tc.tile_pool(name="wop", bufs=1) as wop,
            tc.tile_pool(name="ctin", bufs=3) as ctin,
            tc.tile_pool(name="opsum", bufs=4, space="PSUM") as ops,
            tc.tile_pool(name="ostage", bufs=4) as ost,
        ):
            wo_sb = wop.tile([128, HPC, D], F32R)
            for hh in range(HPC):
                nc.sync.dma_start(out=wo_sb[:, hh, :], in_=wo_r[:, hh, :])

            for st in range(ST):
                ct_sb = ctin.tile([128, HPC, 128], F32R, tag="ctin")
                nc.sync.dma_start(
                    out=ct_sb, in_=ct_r[:, :, st * 128 : (st + 1) * 128]
                )
                for ncol in range(4):
                    psum = ops.tile([128, 512], F32)
                    for hh in range(HPC):
                        nc.tensor.matmul(
                            psum,
                            ct_sb[:, hh, :],
                            wo_sb[:, hh, ncol * 512 : (ncol + 1) * 512],
                            start=(hh == 0),
                            stop=(hh == HPC - 1),
                        )
                    o_sb = ost.tile([128, 512], F32, tag="ostage")
                    nc.scalar.activation(
                        out=o_sb, in_=psum, func=mybir.ActivationFunctionType.Copy
                    )
                    nc.gpsimd.dma_start(
                        out=out[
                            st * 128 : (st + 1) * 128,
                            ncol * 512 : (ncol + 1) * 512,
                        ],
                        in_=o_sb,
                    )


ctpool_tiles = {}

_NC = None


def _get_nc():
    global _NC
    if _NC is None:
        ctpool_tiles.clear()
        _NC = _build_nc()
    return _NC


def _host_prep(input_sequences, Wq, bq, Wk, bk, Wv, bv, Wo, bo):
    """Build per-core input maps."""
    x = np.asarray(input_sequences, dtype=np.float32)
    cm = np.full((128, 896), NEG, dtype=np.float32)
    kk = np.arange(128)[:, None]
    uu = np.arange(896)[None, :]
    cm[kk <= uu - 384] = 0.0

    in_maps = []
    for c in range(8):
        b, g = divmod(c, 2)
        sl = slice(g * DHG, (g + 1) * DHG)
        wq_c = np.ascontiguousarray(
            np.asarray(Wq[:, sl], dtype=np.float32)
            .reshape(KT, 128, HPC, 128).transpose(2, 1, 0, 3).reshape(DHG, D)
        )
        wk_c = np.ascontiguousarray(
            np.asarray(Wk[:, sl], dtype=np.float32)
            .reshape(KT, 128, HPC, 128).transpose(2, 1, 0, 3).reshape(DHG, D)
        )
        wv_c = np.ascontiguousarray(Wv[:, sl], dtype=np.float32)
        wo_c = np.ascontiguousarray(Wo[sl, :], dtype=np.float32)
        in_maps.append({
            "xT": np.ascontiguousarray(x[b].T),
            "wq": wq_c,
            "wk": wk_c,
            "wv": wv_c,
            "wo": wo_c,
            "bqT": np.ascontiguousarray(
                np.asarray(bq[sl], dtype=np.float32).reshape(HPC, 128).T
            ),
            "bkT": np.ascontiguousarray(
                np.asarray(bk[sl], dtype=np.float32).reshape(HPC, 128).T
            ),
            "bvb": np.ascontiguousarray(
                np.broadcast_to(np.asarray(bv[sl], dtype=np.float32), (128, DHG))
            ),
            "cmask": cm,
        })
    return in_maps


def kernel(input_sequences, Wq, bq, Wk, bk, Wv, bv, Wo, bo, _trace=False):
    nc = _get_nc()
    in_maps = _host_prep(input_sequences, Wq, bq, Wk, bk, Wv, bv, Wo, bo)
    res = run_bass_kernel_spmd(nc, in_maps, list(range(8)), trace=_trace)
    bo32 = np.asarray(bo, dtype=np.float32)
    out = np.empty((B, S, D), dtype=np.float32)
    for b in range(B):
        out[b] = res.results[2 * b]["out"] + res.results[2 * b + 1]["out"] + bo32
    if _trace:
        kernel.last_exec_time_ns = res.exec_time_ns
    return out


# revision 19
# speedup vs baseline: 1.2387x; 1.0763x over previous
"""Causal multi-head attention on 8 trn2 NeuronCores.

Problem: B=4, S=2048, D=2048, H=16 heads, head_dim=128, causal softmax,
torch-style Linear projections (W stored [in, out]).

Sharding: core c handles batch b = c//2 and head-group g = c%2
(8 heads = 1024 output columns of Wq/Wk/Wv, 1024 rows of Wo).
Each core produces a partial output [S, D]; host sums the two
head-group partials per batch and adds bo.

Per-core device pipeline (all matmuls fp32r, 1 cycle/row):
  Phase A: from xT (host-pretransposed [D, S]) compute
           Q^T, K^T [1024, S] and V [S, 1024]; spill to DRAM scratch.
  Phase B: per head h: scores^T tiles [128 k, 512 q] = K_h Q_h^T,
           causal mask (additive, precomputed), exp (no max-subtract:
           scores are O(5), fp32 exp is safe), ctx^T accumulation
           C^T = V_h^T-blocks @ P^T, denominators via ones-matmul,
           normalize with reciprocal broadcast (PE outer product).
  Phase C: out_partial = C @ Wo_slice via C^T blocks as lhsT.
"""

import numpy as np

import concourse.bass as bass
import concourse.mybir as mybir
import concourse.tile as tile
from concourse import bacc
from concourse.bass_utils import run_bass_kernel_spmd

B = 4
S = 2048
D = 2048
H = 16
DH = 128
HPC = 8          # heads per core
DHG = HPC * DH   # 1024: head-group width per core
KT = D // 128    # 16 k-tiles over the model dim
ST = S // 128    # 16 s-tiles
QC = S // 512    # 4 q-chunks
SCALE = 1.0 / np.sqrt(DH)
NEG = -1.0e30

F32 = mybir.dt.float32
F32R = mybir.dt.float32r


def _build_nc():
    nc = bacc.Bacc(None, target_bir_lowering=False)

    xT = nc.declare_dram_parameter("xT", [D, S], F32, isOutput=False)
    # wq/wk host-pregathered to [HPC*128, KT*128]: row t*128+p, col n*128+m
    # = Wq[n*128+p, t*128+m] so each head-tile's weights DMA contiguously
    wq = nc.declare_dram_parameter("wq", [DHG, D], F32, isOutput=False)
    wk = nc.declare_dram_parameter("wk", [DHG, D], F32, isOutput=False)
    wv = nc.declare_dram_parameter("wv", [D, DHG], F32, isOutput=False)
    wo = nc.declare_dram_parameter("wo", [DHG, D], F32, isOutput=False)
    bqT = nc.declare_dram_parameter("bqT", [128, HPC], F32, isOutput=False)
    bkT = nc.declare_dram_parameter("bkT", [128, HPC], F32, isOutput=False)
    bvb = nc.declare_dram_parameter("bvb", [128, DHG], F32, isOutput=False)
    cmask = nc.declare_dram_parameter("cmask", [128, 896], F32, isOutput=False)
    out = nc.declare_dram_parameter("out", [S, D], F32, isOutput=True)

    with tile.TileContext(nc) as tc:
        _emit(nc, tc, xT, wq, wk, wv, wo, bqT, bkT, bvb, cmask, out)
    nc.compile()
    return nc


def _emit(nc, tc, xT, wq, wk, wv, wo, bqT, bkT, bvb, cmask, out):
    with (
        tc.tile_pool(name="const", bufs=1) as const,
        tc.tile_pool(name="dram", bufs=1, space="DRAM") as dram,
        tc.tile_pool(name="ct", bufs=4) as ctpool,
    ):
        qt_d = dram.tile([DHG, S], F32R)
        kt_d = dram.tile([DHG, S], F32R)
        v_d = dram.tile([S, DHG], F32R)
        ct_d = dram.tile([DHG, S], F32R)

        cm_sb = const.tile([128, 896], F32)
        nc.sync.dma_start(out=cm_sb, in_=cmask[:, :])
        bq_sb = const.tile([128, HPC], F32)
        nc.sync.dma_start(out=bq_sb, in_=bqT[:, :])
        bk_sb = const.tile([128, HPC], F32)
        nc.sync.dma_start(out=bk_sb, in_=bkT[:, :])
        bv_sb = const.tile([128, DHG], F32)
        nc.sync.dma_start(out=bv_sb, in_=bvb[:, :])
        ones_f32 = const.tile([128, 128], F32)
        nc.vector.memset(ones_f32, 1.0)
        ones128 = const.tile([128, 128], F32R)
        nc.vector.tensor_copy(out=ones128, in_=ones_f32)

        # ---------------- Phase A: projections, spilled to DRAM -------------
        # Two S-halves so xT residency is 8MB instead of 16MB.
        wv_r = wv.bitcast(F32R).rearrange("(n p) m -> p n m", p=128)

        with (
            tc.tile_pool(name="xts", bufs=3) as xtp,
            tc.tile_pool(name="wqk", bufs=3) as wqk,
            tc.tile_pool(name="wvp", bufs=6) as wvp,
            tc.tile_pool(name="apsum", bufs=4, space="PSUM") as aps,
            tc.tile_pool(name="astage", bufs=6) as ast,
        ):
            for sh in range(2):
                s0 = sh * (S // 2)
                # weight tiles prefetched (depth 2) ahead of the bulk xT DMAs
                seq = [(w, b, dst, t)
                       for w, b, dst in ((wq, bq_sb, qt_d), (wk, bk_sb, kt_d))
                       for t in range(HPC)]
                w_tiles = {}

                def w_prefetch(i):
                    if i < len(seq):
                        w, _, _, t = seq[i]
                        w_sb = wqk.tile([128, KT, 128], F32R, tag="wqk",
                                        name=f"w_sb{i % 3}")
                        nc.sync.dma_start(
                            out=w_sb,
                            in_=w[t * 128 : (t + 1) * 128, :]
                            .rearrange("p (n m) -> p n m", m=128)
                            .bitcast(F32R),
                        )
                        w_tiles[i] = w_sb

                w_prefetch(0)
                w_prefetch(1)

                # xT half as two sub-tiles of 8 k-tiles each (bufs=3 lets the
                # next half's first sub-tile prefetch during this half).
                xt_lo = xtp.tile([128, 8, S // 2], F32R, tag="xts")
                xt_hi = xtp.tile([128, 8, S // 2], F32R, tag="xts")

                def xt_blk(kd):
                    t = xt_lo if kd < 8 else xt_hi
                    return t[:, kd % 8, :]

                for kd in range(KT):
                    nc.sync.dma_start(
                        out=xt_blk(kd),
                        in_=xT[kd * 128 : (kd + 1) * 128, s0 : s0 + S // 2].bitcast(F32R),
                    )

                # Q^T and K^T: psum[dh 128, s 512] = sum_kd Wblk^T @ xTblk
                for i, (w, b_sb, dst, t) in enumerate(seq):
                    w_sb = w_tiles.pop(i)
                    w_prefetch(i + 2)
                    for sc in range(2):
                        psum = aps.tile([128, 512], F32)
                        for kd in range(KT):
                            nc.tensor.matmul(
                                psum,
                                w_sb[:, kd, :],
                                xt_blk(kd)[:, sc * 512 : (sc + 1) * 512],
                                start=(kd == 0),
                                stop=(kd == KT - 1),
                            )
                        stg = ast.tile([128, 512], F32R, tag="astage")
                        nc.vector.tensor_scalar_add(
                            out=stg, in0=psum, scalar1=b_sb[:, t : t + 1]
                        )
                        nc.gpsimd.dma_start(
                            out=dst[
                                t * 128 : (t + 1) * 128,
                                s0 + sc * 512 : s0 + (sc + 1) * 512,
                            ],
                            in_=stg,
                        )

                # V: psum[s 128, dh 512] = sum_kd xTblk^T @ Wvblk.
                # wv streamed per k-tile; 4 s-tiles accumulate concurrently.
                for t2 in range(2):
                    for sb in range(2):  # blocks of 4 s-tiles
                        psums = [
                            aps.tile([128, 512], F32, tag="apsum", name=f"vps{si}")
                            for si in range(4)
                        ]
                        for kd in range(KT):
                            wv_sb = wvp.tile([128, 512], F32R, tag="wvp")
                            nc.sync.dma_start(
                                out=wv_sb,
                                in_=wv_r[:, kd, t2 * 512 : (t2 + 1) * 512],
                            )
                            for si in range(4):
                                st = sb * 4 + si
                                nc.tensor.matmul(
                                    psums[si],
                                    xt_blk(kd)[:, st * 128 : (st + 1) * 128],
                                    wv_sb,
                                    start=(kd == 0),
                                    stop=(kd == KT - 1),
                                )
                        for si in range(4):
                            st = sb * 4 + si
                            stg = ast.tile([128, 512], F32R, tag="astage")
                            nc.vector.tensor_tensor(
                                out=stg,
                                in0=psums[si],
                                in1=bv_sb[:, t2 * 512 : (t2 + 1) * 512],
                                op=mybir.AluOpType.add,
                            )
                            nc.gpsimd.dma_start(
                                out=v_d[
                                    s0 + st * 128 : s0 + (st + 1) * 128,
                                    t2 * 512 : (t2 + 1) * 512,
                                ],
                                in_=stg,
                            )

        # ---------------- Phase B: per-head attention ------------------------
        v_r = v_d[:, :].rearrange("(n p) d -> p n d", p=128)
        wo_r = wo.bitcast(F32R).rearrange("(n p) m -> p n m", p=128)
        wop_cm = tc.tile_pool(name="wop", bufs=1)
        wop = wop_cm.__enter__()
        with (
            tc.tile_pool(name="qkv", bufs=2) as qkv,
            tc.tile_pool(name="ptile", bufs=6) as ppool,
            tc.tile_pool(name="msk", bufs=3) as mpool,
            tc.tile_pool(name="rcp", bufs=2) as rcpool,
            tc.tile_pool(name="pscore", bufs=3, space="PSUM") as pscore,
            tc.tile_pool(name="pctx", bufs=2, space="PSUM") as pctx,
            tc.tile_pool(name="psum2", bufs=2, space="PSUM") as psums,
        ):
            wo_sb = wop.tile([128, HPC, D], F32R)
            for h in range(HPC):
                qt_sb = qkv.tile([128, S], F32R, tag="qt")
                nc.sync.dma_start(out=qt_sb, in_=qt_d[h * 128 : (h + 1) * 128, :])
                kt_sb = qkv.tile([128, S], F32R, tag="kt")
                nc.sync.dma_start(out=kt_sb, in_=kt_d[h * 128 : (h + 1) * 128, :])
                v_sb = qkv.tile([128, ST, 128], F32R, tag="v")
                nc.sync.dma_start(out=v_sb, in_=v_r[:, :, h * 128 : (h + 1) * 128])
                # spread the 8MB Wo load through phase B on the idle sync queue
                nc.sync.dma_start(out=wo_sb[:, h, :], in_=wo_r[:, h, :])

                for qc in range(QC):
                    nkt = 4 * qc + 4
                    # diagonal tiles first: their longer PE->DVE->ACT chains
                    # start early and overlap with the full tiles' stream
                    order = list(range(4 * qc, nkt)) + list(range(4 * qc))
                    psum_c = pctx.tile([128, 512], F32)
                    psum_s = psums.tile([128, 512], F32)

                    def scores(kt_i):
                        # diagonal tile j has valid columns only at qq >= 128j:
                        # compute just that [128, 512-128j] strip
                        j = kt_i - 4 * qc
                        off = 128 * j if j > 0 else 0
                        ps_t = pscore.tile([128, 512], F32, tag="ps_t")
                        nc.tensor.matmul(
                            ps_t[:, off:],
                            kt_sb[:, kt_i * 128 : (kt_i + 1) * 128],
                            qt_sb[:, qc * 512 + off : (qc + 1) * 512],
                            start=True,
                            stop=True,
                        )
                        p_t = ppool.tile([128, 512], F32R, tag="p_t")
                        if j >= 0:
                            msk = mpool.tile([128, 512], F32, tag="msk")
                            nc.vector.tensor_tensor(
                                out=msk[:, off:],
                                in0=ps_t[:, off:],
                                in1=cm_sb[:, 384 : 896 - off],
                                op=mybir.AluOpType.add,
                            )
                            src = msk
                        else:
                            src = ps_t
                        nc.scalar.activation(
                            out=p_t[:, off:],
                            in_=src[:, off:],
                            func=mybir.ActivationFunctionType.Exp,
                            scale=float(SCALE),
                        )
                        return p_t, off

                    def ctx(idx, kt_i, p_t, off):
                        nc.tensor.matmul(
                            psum_c[:, off:],
                            v_sb[:, kt_i, :],
                            p_t[:, off:],
                            start=(idx == 0),
                            stop=(idx == nkt - 1),
                        )
                        # every psum_s row accumulates the per-q denominator
                        nc.tensor.matmul(
                            psum_s[:, off:],
                            ones128,
                            p_t[:, off:],
                            start=(idx == 0),
                            stop=(idx == nkt - 1),
                        )

                    # software-pipeline scores/exp ahead of ctx by one tile
                    prev = None
                    for idx, kt_i in enumerate(order):
                        p_t, off = scores(kt_i)
                        if prev is not None:
                            ctx(idx - 1, prev[0], prev[1], prev[2])
                        prev = (kt_i, p_t, off)
                    ctx(nkt - 1, prev[0], prev[1], prev[2])

                    recip = rcpool.tile([128, 512], F32, tag="rcp")
                    nc.vector.reciprocal_approx_fast(out=recip, in_=psum_s)
                    ct = ctpool.tile([128, 512], F32R, tag="ct")
                    nc.vector.tensor_tensor(
                        out=ct,
                        in0=psum_c,
                        in1=recip,
                        op=mybir.AluOpType.mult,
                    )
                    nc.gpsimd.dma_start(
                        out=ct_d[h * 128 : (h + 1) * 128, qc * 512 : (qc + 1) * 512],
                        in_=ct,
                    )

        # ---------------- Phase C: output projection -------------------------
        ct_r = ct_d[:, :].rearrange("(n p) m -> p n m", p=128)
        with (
            tc.tile_pool(name="ctin", bufs=3) as ctin,
            tc.tile_pool(name="opsum", bufs=4, space="PSUM") as ops,
            tc.tile_pool(name="ostage", bufs=4) as ost,
        ):
            for st in range(ST):
                ct_sb = ctin.tile([128, HPC, 128], F32R, tag="ctin")
                nc.sync.dma_start(
                    out=ct_sb, in_=ct_r[:, :, st * 128 : (st + 1) * 128]
                )
                for ncol in range(4):
                    psum = ops.tile([128, 512], F32)
                    for hh in range(HPC):
                        nc.tensor.matmul(
                            psum,
                            ct_sb[:, hh, :],
                            wo_sb[:, hh, ncol * 512 : (ncol + 1) * 512],
                            start=(hh == 0),
                            stop=(hh == HPC - 1),
                        )
                    o_sb = ost.tile([128, 512], F32, tag="ostage")
                    nc.scalar.activation(
                        out=o_sb, in_=psum, func=mybir.ActivationFunctionType.Copy
                    )
                    nc.gpsimd.dma_start(
                        out=out[
                            st * 128 : (st + 1) * 128,
                            ncol * 512 : (ncol + 1) * 512,
                        ],
                        in_=o_sb,
                    )
        wop_cm.__exit__(None, None, None)


ctpool_tiles = {}

_NC = None


def _get_nc():
    global _NC
    if _NC is None:
        ctpool_tiles.clear()
        _NC = _build_nc()
    return _NC


def _host_prep(input_sequences, Wq, bq, Wk, bk, Wv, bv, Wo, bo):
    """Build per-core input maps."""
    x = np.asarray(input_sequences, dtype=np.float32)
    cm = np.full((128, 896), NEG, dtype=np.float32)
    kk = np.arange(128)[:, None]
    uu = np.arange(896)[None, :]
    cm[kk <= uu - 384] = 0.0

    in_maps = []
    for c in range(8):
        b, g = divmod(c, 2)
        sl = slice(g * DHG, (g + 1) * DHG)
        wq_c = np.ascontiguousarray(
            np.asarray(Wq[:, sl], dtype=np.float32)
            .reshape(KT, 128, HPC, 128).transpose(2, 1, 0, 3).reshape(DHG, D)
        )
        wk_c = np.ascontiguousarray(
            np.asarray(Wk[:, sl], dtype=np.float32)
            .reshape(KT, 128, HPC, 128).transpose(2, 1, 0, 3).reshape(DHG, D)
        )
        wv_c = np.ascontiguousarray(Wv[:, sl], dtype=np.float32)
        wo_c = np.ascontiguousarray(Wo[sl, :], dtype=np.float32)
        in_maps.append({
            "xT": np.ascontiguousarray(x[b].T),
            "wq": wq_c,
            "wk": wk_c,
            "wv": wv_c,
            "wo": wo_c,
            "bqT": np.ascontiguousarray(
                np.asarray(bq[sl], dtype=np.float32).reshape(HPC, 128).T
            ),
            "bkT": np.ascontiguousarray(
                np.asarray(bk[sl], dtype=np.float32).reshape(HPC, 128).T
            ),
            "bvb": np.ascontiguousarray(
                np.broadcast_to(np.asarray(bv[sl], dtype=np.float32), (128, DHG))
            ),
            "cmask": cm,
        })
    return in_maps


def kernel(input_sequences, Wq, bq, Wk, bk, Wv, bv, Wo, bo, _trace=False):
    nc = _get_nc()
    in_maps = _host_prep(input_sequences, Wq, bq, Wk, bk, Wv, bv, Wo, bo)
    res = run_bass_kernel_spmd(nc, in_maps, list(range(8)), trace=_trace)
    bo32 = np.asarray(bo, dtype=np.float32)
    out = np.empty((B, S, D), dtype=np.float32)
    for b in range(B):
        out[b] = res.results[2 * b]["out"] + res.results[2 * b + 1]["out"] + bo32
    if _trace:
        kernel.last_exec_time_ns = res.exec_time_ns
    return out
